# revision 35
# baseline (speedup 1.0000x reference)
"""Trainium2 Bass kernel for nn_BaselineBlock_SCA_Modulated.

Sharding: 8 cores = 2 batch x 4 D-slabs of 16 planes each. Halo planes are
staged host-side (zero planes at global D edges); all cores run one SPMD
program.

Phase A (per plane): transpose-DMA load -> LN1 stats on DVE (poly+Newton
rsqrt; Act keeps the gelu table loaded for the whole kernel) -> normalize in
transposed layout -> PE transpose-back -> Act copies into a padded fp8 conv
tile (lower 64 partitions = plane, upper = +1 column shift) -> modulated
3x3x3 depthwise conv fused with pw1 as 9 fp8 DoubleRow quad-matmuls (up to
4 taps each: partition pairing = +1 column, j-dim pairing = arbitrary
stride) + 1 boundary-correction quad -> Act gelu with fused per-channel
(s*demod) scale / mod_b bias / pooling accum -> fp8 xg to DRAM.

AllReduce pooled stats -> SCA gate, folded into pw3 weights on device.

Phase B (per plane): fp8 DoubleRow pw3/pw4/pw5 (dead j-tile via stride-0
rhs + zero weights); y kept bf16 (only feeds LN2); LN2 mean-subtraction
folded into pw4 via a 65th rhs row; final residual accumulated in PSUM via
an exact bf16 hi/lo identity matmul so out = inp + beta*x3 + gamma*x5 + bias
drains through a single Act Identity. fp8 weights use power-of-2 scaling to
stay out of the subnormal range; the inverse scales ride the Act scale APs.
"""
import numpy as np
import ml_dtypes

C, DW, SD = 64, 128, 512
D, H, W = 64, 64, 64
NPL = 16              # output planes per core
NHALO = NPL + 2       # input planes incl halo
PW = 66               # padded row width
PSZ = PW * PW + 2     # padded plane size + slack
HWC = H * W           # 4096
NCH = HWC // 128      # 32 transpose chunks / plane
bf = ml_dtypes.bfloat16
f8 = ml_dtypes.float8_e4m3

S3 = 3                # pw3 (y path) fp8 weight scale exponent (build-time)

# rsqrt seed poly on [0.125, 8]: y0 = RC0 + RC1*v + RC2/v (2 Newton steps)
RC = (0.66086714, -0.05091159, 0.33043357)

_CACHE = {}


def _build():
    import bass_rust
    import concourse.bacc as bacc
    import concourse.mybir as mybir
    import concourse.tile as tile
    from concourse.mybir import ActivationFunctionType as AF, AluOpType as ALU

    BF = mybir.dt.bfloat16
    F32 = mybir.dt.float32
    FP8 = mybir.dt.float8e4
    AX = mybir.AxisListType
    DR = mybir.MatmulPerfMode.DoubleRow

    nc = bacc.Bacc("TRN2", target_bir_lowering=False, debug=False, num_devices=8)

    dram = {}
    def din(name, shape, dt):
        dram[name] = nc.dram_tensor(name, shape, dt, kind="ExternalInput")
        return dram[name]

    inp_t = din("inp_t", [NHALO, C, HWC], BF)
    inp_hl = din("inp_hl", [NPL, 128, HWC], BF)
    wq_i = din("wq", [128, 9, 2, 128], FP8)          # 3 kd x 3 q quads
    wc_i = din("wc", [9, 2, NPL, 128], FP8)
    ind_i = din("ind", [9, HWC], FP8)
    sd_i = din("sd", [128, 1], F32)
    modb_i = din("modb", [128, 1], F32)
    w3T_i = din("w3T", [128, 64], BF)
    bmap_i = din("bmap", [128, 64], BF)              # beta[c] along free dim
    scawT_i = din("scawT", [128, 128], BF)
    scab_i = din("scab", [128, 1], F32)
    beta3_i = din("beta3", [64, 1], F32)             # beta * 2^-S3
    w48_i = din("w48", [65, 2, 128], FP8)
    b4_i = din("b4", [128, 1], F32)
    s4inv_i = din("s4inv", [128, 1], F32)            # 2^-S4
    w58_i = din("w58", [128, 2, 64], FP8)
    ident2_i = din("ident2", [128, 64], BF)          # [2^S5*I; 2^S5*I]
    identB_i = din("identB", [128, 64], BF)          # [diag(2^S3/beta); same]
    b5g_i = din("b5g", [64, 1], F32)                 # (b5*gamma) * 2^S5
    s5inv_i = din("s5inv", [64, 1], F32)             # 2^-S5
    s5w3_i = din("s5w3", [128, 1], F32)              # gate-fold scale 2^S5
    i128_i = din("i128", [128, 128], BF)
    i64b_i = din("i64b", [64, 64], BF)
    out_d = nc.dram_tensor("out", [NPL, C, HWC], F32, kind="ExternalOutput")

    xg_scr = nc.dram_tensor("xg_scr", [NPL, 128, HWC], FP8,
                            kind="ExternalOutput" if _CACHE.get("dbg") else "Internal")
    dbg_y = (nc.dram_tensor("dbg_y", [NPL, 64, HWC], F32, kind="ExternalOutput")
             if _CACHE.get("dbg") else None)
    dbg_yt = (nc.dram_tensor("dbg_yt", [NPL, 65, HWC], F32, kind="ExternalOutput")
              if _CACHE.get("dbg") else None)
    dbg_g = (nc.dram_tensor("dbg_g", [128, 1], F32, kind="ExternalOutput")
             if _CACHE.get("dbg") else None)
    mrv_scr = nc.dram_tensor("mrv_scr", [NPL, 128, NCH], FP8)
    cc_a = nc.dram_tensor("cc_a", [128, 1], F32)
    cc_b = nc.dram_tensor("cc_b", [128, 1], F32)

    V = bass_rust.VecI64Pair

    def sview(t, np_, off, dims):
        """Strided free view of tile t: partitions [0, np_), free dims =
        [(stride, size), ...] at element offset off."""
        v = t[0:np_, off:off + 1]
        c = v.copy()
        pap = v.ap.to_list()[0]
        c.ap = V([pap] + [[s, n] for s, n in dims])
        return c

    from contextlib import ExitStack
    with tile.TileContext(nc) as tc, ExitStack() as stk:
        cpool = stk.enter_context(tc.tile_pool(name="const", bufs=1))
        rpool = stk.enter_context(tc.tile_pool(name="ring", bufs=1))
        wpool = stk.enter_context(tc.tile_pool(name="work", bufs=2))
        spool = stk.enter_context(tc.tile_pool(name="small", bufs=3))
        bpool = stk.enter_context(tc.tile_pool(name="b", bufs=2))
        psA = stk.enter_context(tc.tile_pool(name="psA", bufs=2, space="PSUM"))
        psB = stk.enter_context(tc.tile_pool(name="psB", bufs=2, space="PSUM"))

        def const(name, shape, dt):
            t = cpool.tile(shape, dt, tag=name, name=name)
            nc.sync.dma_start(t[:], dram[name][:])
            return t

        wq = const("wq", [128, 9, 2, 128], FP8)
        wc = const("wc", [9, 2, NPL, 128], FP8)
        ind = const("ind", [9, HWC], FP8)
        sd = const("sd", [128, 1], F32)
        modb = const("modb", [128, 1], F32)
        w3T = const("w3T", [128, 64], BF)
        bmap = const("bmap", [128, 64], BF)
        scawT = const("scawT", [128, 128], BF)
        scab = const("scab", [128, 1], F32)
        beta3 = const("beta3", [64, 1], F32)
        w48 = const("w48", [65, 2, 128], FP8)
        b4 = const("b4", [128, 1], F32)
        s4inv = const("s4inv", [128, 1], F32)
        w58 = const("w58", [128, 2, 64], FP8)
        ident2 = const("ident2", [128, 64], BF)
        identB = const("identB", [128, 64], BF)
        b5g = const("b5g", [64, 1], F32)
        s5inv = const("s5inv", [64, 1], F32)
        s5w3 = const("s5w3", [128, 1], F32)
        i128 = const("i128", [128, 128], BF)
        i64b = const("i64b", [64, 64], BF)

        pools = cpool.tile([128, NPL * 8], F32, tag="pools")
        w3g8 = cpool.tile([128, 2, 64], FP8, tag="w3g8", name="w3g8")
        w3gb8 = cpool.tile([128, 2, 64], FP8, tag="w3gb8", name="w3gb8")
        nc.vector.memset(w3g8[:], 0.0)
        nc.vector.memset(w3gb8[:], 0.0)

        NS = 4
        t1s = [rpool.tile([128, PSZ], FP8, tag=f"t1_{i}", name=f"t1_{i}")
               for i in range(NS)]
        for i in range(NS):
            nc.gpsimd.memset(t1s[i][:], 0.0)

        def rsqrt_chain(var, tag):
            """var [128, 32] f32 (clamped to [0.125, 8]) -> ~var^-0.5."""
            r = spool.tile([128, NCH], F32, tag=f"{tag}_r")
            nc.vector.reciprocal(r[:], var[:])
            a1 = spool.tile([128, NCH], F32, tag=f"{tag}_a1")
            nc.vector.tensor_scalar(a1[:], var[:], RC[1], RC[0],
                                    op0=ALU.mult, op1=ALU.add)
            y0 = spool.tile([128, NCH], F32, tag=f"{tag}_y0")
            nc.vector.scalar_tensor_tensor(y0[:], r[:], RC[2], a1[:],
                                           op0=ALU.mult, op1=ALU.add)
            y = y0
            for it in range(2):
                t1_ = spool.tile([128, NCH], F32, tag=f"{tag}_t{it}")
                nc.vector.tensor_mul(t1_[:], y[:], y[:])
                nc.vector.tensor_mul(t1_[:], t1_[:], var[:])
                nc.vector.tensor_scalar(t1_[:], t1_[:], -0.5, 1.5,
                                        op0=ALU.mult, op1=ALU.add)
                yn = spool.tile([128, NCH], F32, tag=f"{tag}_y{it}")
                nc.vector.tensor_mul(yn[:], y[:], t1_[:])
                y = yn
            return y

        def ln_stats(xT, tag):
            """xT [128, 32, 64] bf16 -> (rvb bf16, mrv f32) [128, 32]."""
            sq = wpool.tile([128, NCH, 64], BF, tag=f"{tag}_sq", bufs=2)
            nc.vector.tensor_mul(sq[:], xT[:], xT[:])
            msum = spool.tile([128, NCH], BF, tag=f"{tag}_ms")
            qsum = spool.tile([128, NCH], BF, tag=f"{tag}_qs")
            with nc.allow_low_precision(reason="bf16 LN stats, 0.4% ok"):
                nc.vector.tensor_reduce(msum[:], xT[:], axis=AX.X, op=ALU.add)
                nc.vector.tensor_reduce(qsum[:], sq[:], axis=AX.X, op=ALU.add)
            q63 = spool.tile([128, NCH], F32, tag=f"{tag}_q63")
            nc.vector.tensor_scalar(q63[:], qsum[:], 1.0 / 63.0, None,
                                    op0=ALU.mult)
            msq = spool.tile([128, NCH], F32, tag=f"{tag}_msq")
            nc.vector.tensor_mul(msq[:], msum[:], msum[:])
            var = spool.tile([128, NCH], F32, tag=f"{tag}_var")
            nc.vector.scalar_tensor_tensor(var[:], msq[:], -1.0 / (64.0 * 63.0),
                                           q63[:], op0=ALU.mult, op1=ALU.add)
            nc.vector.tensor_scalar(var[:], var[:], 0.125, 8.0,
                                    op0=ALU.max, op1=ALU.min)
            rv = rsqrt_chain(var, tag)
            mrv = spool.tile([128, NCH], F32, tag=f"{tag}_mrv")
            nc.vector.scalar_tensor_tensor(mrv[:], msum[:], 1.0 / 64.0, rv[:],
                                           op0=ALU.mult, op1=ALU.mult)
            rvb = spool.tile([128, NCH], BF, tag=f"{tag}_rvb")
            nc.vector.tensor_copy(rvb[:], rv[:])
            return rvb, mrv

        # ---------------- PASS 1 ----------------
        def ln1_plane(p):
            t1 = t1s[p % NS]
            xT = wpool.tile([128, NCH, 64], BF, tag="xT", bufs=3)
            nc.sync.dma_start_transpose(xT[:], inp_t[p])
            rvb, mrv = ln_stats(xT, "l1")
            mrvb = spool.tile([128, NCH], BF, tag="l1_mrvb")
            nc.vector.tensor_copy(mrvb[:], mrv[:])
            xln = wpool.tile([128, NCH, 64], BF, tag="xln", bufs=3)
            nc.vector.tensor_mul(
                xln[:], xT[:], rvb[:].unsqueeze(2).broadcast_to([128, NCH, 64]))
            nc.vector.tensor_sub(
                xln[:], xln[:], mrvb[:].unsqueeze(2).broadcast_to([128, NCH, 64]))
            for q in range(4):
                pst = psA.tile([64, 1024], BF, tag="tr", bufs=4)
                for g in range(8):
                    nc.tensor.transpose(
                        pst[:, g * 128:(g + 1) * 128], xln[:, 8 * q + g, :],
                        i128[:])
                dst = t1[0:64, 0:PW * PW].rearrange(
                    "p (r w) -> p r w", w=PW)[:, 1 + 16 * q:1 + 16 * (q + 1), 1:65]
                src = pst[:].rearrange("p (r w) -> p r w", w=64)
                nc.scalar.activation(dst, src, AF.Copy)
            # upper half = same plane shifted one column (U[o] = L[o+1])
            nc.sync.dma_start(t1[64:128, 0:PSZ - 1], t1[0:64, 1:PSZ])

        # quad table: (q, j0 row off, j0 col off, j-delta)
        QUADS = ((0, 0, 0, 66), (1, 0, 2, 130), (2, 1, 2, 66))

        def conv_plane(i):
            slots = [t1s[(i + kd) % NS] for kd in range(3)]
            xg = bpool.tile([128, HWC], FP8, tag="xg")
            for cb in range(8):
                ps = psB.tile([128, 512], F32, tag="conv")
                for h2 in range(2):
                    psl = ps[:, h2 * 256:(h2 + 1) * 256]
                    first = True
                    for kd in range(3):
                        for (q, ro, co, dj) in QUADS:
                            off = (8 * cb + 4 * h2 + ro) * PW + co
                            rhs = sview(slots[kd], 128, off,
                                        [(dj, 2), (PW, 4), (1, 64)])
                            nc.tensor.matmul(psl, wq[:, 3 * kd + q], rhs,
                                             start=first, stop=False,
                                             perf_mode=DR)
                            first = False
                    rhs_c = sview(ind, 9, cb * 512 + h2 * 256,
                                  [(0, 2), (1, 256)])
                    nc.tensor.matmul(psl, wc[:, :, i, :], rhs_c,
                                     start=False, stop=True, perf_mode=DR)
                nc.scalar.activation(
                    xg[:, cb * 512:(cb + 1) * 512], ps[:], AF.Gelu,
                    bias=modb[:], scale=sd[:],
                    accum_out=pools[:, i * 8 + cb:i * 8 + cb + 1])
            nc.sync.dma_start(xg_scr[i], xg[:])

        for p in range(NHALO):
            ln1_plane(p)
            if p >= 2:
                conv_plane(p - 2)

        # ---------------- pooled -> gate -> pw3 weights ----------------
        pooled = cpool.tile([128, 1], F32, tag="pooled")
        nc.vector.tensor_reduce(pooled[:], pools[:], axis=AX.X, op=ALU.add)
        nc.sync.dma_start(cc_a[:], pooled[:])
        nc.gpsimd.collective_compute(
            "AllReduce", ALU.add,
            replica_groups=[[0, 1, 2, 3], [4, 5, 6, 7]],
            ins=[cc_a[:]], outs=[cc_b[:]])
        pooled2f = cpool.tile([128, 1], F32, tag="pooled2f", name="pooled2f")
        nc.sync.dma_start(pooled2f[:], cc_b[:])
        pooled2 = cpool.tile([128, 1], BF, tag="pooled2", name="pooled2")
        nc.vector.tensor_copy(pooled2[:], pooled2f[:])
        psg = psB.tile([128, 1], F32, tag="conv")
        nc.tensor.matmul(psg[:], scawT[:], pooled2[:], start=True, stop=True)
        gate = cpool.tile([128, 1], F32, tag="gatev")
        nc.scalar.activation(gate[:], psg[:], AF.Identity, bias=scab[:])
        if dbg_g is not None:
            nc.sync.dma_start(dbg_g[:], gate[:])
        w3gp = cpool.tile([128, 64], BF, tag="w3gp")
        nc.vector.tensor_scalar(w3gp[:], w3T[:], gate[:], float(2.0 ** S3),
                                op0=ALU.mult, op1=ALU.mult)
        nc.vector.tensor_copy(w3g8[:, 0, :], w3gp[:])
        w3gb = cpool.tile([128, 64], BF, tag="w3gb")
        nc.vector.tensor_scalar(w3gb[:], w3T[:], gate[:], s5w3[:],
                                op0=ALU.mult, op1=ALU.mult)
        nc.vector.tensor_mul(w3gb[:], w3gb[:], bmap[:])
        nc.vector.tensor_copy(w3gb8[:, 0, :], w3gb[:])

        # ---------------- PASS 2 (3-stage software pipeline) ----------------
        p2state = {}

        def pass2_s1(i):
            xg8 = bpool.tile([128, HWC], FP8, tag="xg2in", bufs=3)
            nc.sync.dma_start(xg8[:], xg_scr[i])
            ihl = bpool.tile([128, HWC], BF, tag="ihl", bufs=3)
            nc.sync.dma_start(ihl[:], inp_hl[i])
            ybf = bpool.tile([64, HWC], BF, tag="ybf", bufs=3)
            p2state[i] = (xg8, ihl, ybf)
            for cb in range(8):
                sl = slice(cb * 512, (cb + 1) * 512)
                ps3 = psB.tile([64, 512], F32, tag="conv")
                for h2 in range(2):
                    rhs = sview(xg8, 128, cb * 512 + h2 * 256,
                                [(0, 2), (1, 256)])
                    nc.tensor.matmul(ps3[:, h2 * 256:(h2 + 1) * 256], w3g8[:],
                                     rhs, start=True, stop=True,
                                     perf_mode=DR)
                nc.vector.scalar_tensor_tensor(
                    ybf[:, sl], ps3[:], beta3[:], ihl[0:64, sl],
                    op0=ALU.mult, op1=ALU.add)
        def pass2_s2(i):
            xg8, ihl, ybf = p2state[i]
            # LN2 stats via transposed copies
            yTs = wpool.tile([128, NCH, 64], BF, tag="yTs", bufs=3)
            for hf in range(2):
                psT = psA.tile([128, 1024], BF, tag="tr", bufs=4)
                for g in range(16):
                    nc.tensor.transpose(
                        psT[:, g * 64:(g + 1) * 64],
                        ybf[:, (16 * hf + g) * 128:(16 * hf + g + 1) * 128],
                        i64b[:])
                nc.vector.tensor_copy(
                    yTs[:, 16 * hf:16 * (hf + 1), :],
                    psT[:].rearrange("p (g c) -> p g c", c=64))
            rvb2, mrv2 = ln_stats(yTs, "l2")
            mrv28 = spool.tile([128, NCH], FP8, tag="mrv28")
            nc.vector.tensor_copy(mrv28[:], mrv2[:])
            ytn = wpool.tile([128, NCH, 64], BF, tag="ytn", bufs=3)
            nc.vector.tensor_mul(
                ytn[:], yTs[:], rvb2[:].unsqueeze(2).broadcast_to([128, NCH, 64]))
            ytil = bpool.tile([65, HWC], FP8, tag="ytil", bufs=3)
            # row 64 = mrv2 shuffled to pixel order (q = c*128 + p), via DRAM
            nc.sync.dma_start(mrv_scr[i], mrv28[:])
            src = mrv_scr[i].copy()
            src.ap = V([[1, NCH], [NCH, 128]])
            dst_off = ytil[64:65, 0:1]
            dstc = dst_off.copy()
            dstc.ap = V([dst_off.ap.to_list()[0], [128, NCH], [1, 128]])
            nc.sync.dma_start(dstc, src)
            for q in range(4):
                psY = psA.tile([64, 1024], BF, tag="tr", bufs=4)
                for g in range(8):
                    nc.tensor.transpose(
                        psY[:, g * 128:(g + 1) * 128], ytn[:, 8 * q + g, :],
                        i128[:])
                dst = ytil[0:64, q * 1024:(q + 1) * 1024]
                nc.scalar.activation(dst, psY[:], AF.Copy)
            if dbg_y is not None:
                yf = bpool.tile([64, HWC], F32, tag="dbgyf", bufs=1)
                nc.vector.tensor_copy(yf[:], ybf[:])
                nc.sync.dma_start(dbg_y[i], yf[:])
                ytf = bpool.tile([65, HWC], F32, tag="dbgytf", bufs=1)
                nc.vector.tensor_copy(ytf[:], ytil[:])
                nc.sync.dma_start(dbg_yt[i], ytf[:])
            p2state[i] = (xg8, ihl, ybf, ytil)

        def pass2_s3(i):
            xg8, ihl, ybf, ytil = p2state.pop(i)
            xg2 = bpool.tile([128, HWC], FP8, tag="xg2")
            for hf in range(2):
                outp = bpool.tile([64, 2048], F32, tag="outp")
                for cq in range(4):
                    cb = 4 * hf + cq
                    sl = slice(cb * 512, (cb + 1) * 512)
                    lsl = slice(cq * 512, (cq + 1) * 512)
                    ps4 = psB.tile([128, 512], F32, tag="mm45")
                    for h2 in range(2):
                        rhs4 = sview(ytil, 65, cb * 512 + h2 * 256,
                                     [(0, 2), (1, 256)])
                        nc.tensor.matmul(ps4[:, h2 * 256:(h2 + 1) * 256],
                                         w48[:], rhs4, start=True, stop=True,
                                         perf_mode=DR)
                    nc.scalar.activation(xg2[:, sl], ps4[:], AF.Gelu,
                                         bias=b4[:], scale=s4inv[:])
                    ps5 = psB.tile([64, 512], F32, tag="mm45")
                    nc.tensor.matmul(ps5[:], ident2[:], ihl[:, sl],
                                     start=True, stop=False)
                    for h2 in range(2):
                        hsl = slice(h2 * 256, (h2 + 1) * 256)
                        rhs5 = sview(xg2, 128, cb * 512 + h2 * 256,
                                     [(0, 2), (1, 256)])
                        nc.tensor.matmul(ps5[:, hsl], w58[:], rhs5,
                                         start=False, stop=False, perf_mode=DR)
                        rhs3b = sview(xg8, 128, cb * 512 + h2 * 256,
                                      [(0, 2), (1, 256)])
                        nc.tensor.matmul(ps5[:, hsl], w3gb8[:], rhs3b,
                                         start=False, stop=True,
                                         perf_mode=DR)
                    nc.scalar.activation(outp[:, lsl], ps5[:], AF.Identity,
                                         bias=b5g[:], scale=s5inv[:])
                nc.sync.dma_start(
                    out_d[i][:, hf * 2048:(hf + 1) * 2048], outp[:])

        for ii in range(NPL + 3):
            if ii < NPL:
                pass2_s1(ii)
            if 2 <= ii <= NPL + 1:
                pass2_s2(ii - 2)
            if ii >= 3:
                pass2_s3(ii - 3)

    nc.compile()
    return nc


def _host_prep(inputs):
    inp = np.asarray(inputs["inp"], np.float32)
    style = np.asarray(inputs["style_vector"], np.float32)
    w1 = np.asarray(inputs["w1"], np.float32)
    b1 = np.asarray(inputs["b1"], np.float32)
    mod_w = np.asarray(inputs["mod_w"], np.float32)
    mod_b = np.asarray(inputs["mod_b"], np.float32)
    style_w = np.asarray(inputs["style_w"], np.float32)
    style_b = np.asarray(inputs["style_b"], np.float32)
    sca_w = np.asarray(inputs["sca_w"], np.float32)
    sca_b = np.asarray(inputs["sca_b"], np.float32)
    w3 = np.asarray(inputs["w3"], np.float32)
    b3 = np.asarray(inputs["b3"], np.float32)
    w4 = np.asarray(inputs["w4"], np.float32)
    b4 = np.asarray(inputs["b4"], np.float32)
    w5 = np.asarray(inputs["w5"], np.float32)
    b5 = np.asarray(inputs["b5"], np.float32)
    ln1_w = np.asarray(inputs["ln1_w"], np.float32).reshape(C)
    ln1_b = np.asarray(inputs["ln1_b"], np.float32).reshape(C)
    ln2_w = np.asarray(inputs["ln2_w"], np.float32).reshape(C)
    ln2_b = np.asarray(inputs["ln2_b"], np.float32).reshape(C)
    beta = np.asarray(inputs["beta"], np.float32).reshape(C)
    gamma = np.asarray(inputs["gamma"], np.float32).reshape(C)

    # style modulation (exact, host fp32)
    s = style @ style_w.T + style_b                     # [B, DW]
    k2 = (mod_w ** 2).sum(axis=(1, 2, 3, 4))            # [DW]
    demod = 1.0 / np.sqrt(k2[None] * s * s + 1e-8)      # [B, DW]
    sdv = s * demod                                     # [B, DW]

    W1t = w1 * ln1_w[None, :]                           # [DW, C]
    b1e = b1 + w1 @ ln1_b
    wdw = mod_w[:, 0]                                   # [DW, 3,3,3]

    # conv quad weights: wq[3*kd+q][k, j, m]
    QT = (((0, 0), (1, 0)), ((0, 2), (2, 0)), ((1, 2), (2, 2)))
    wq = np.zeros((9, 128, 2, 128), np.float32)  # transposed to [128,9,2,128] below
    for kd in range(3):
        for q in range(3):
            for j in range(2):
                kh, kw0 = QT[q][j]
                for half in range(2):
                    kw = kw0 + half
                    if kw > 2:
                        continue
                    blk = (W1t * wdw[:, kd, kh, kw][:, None]).T  # [C, DW]
                    wq[3 * kd + q, 64 * half:64 * (half + 1), j, :] = blk

    def Ssum(cd, ch, cw):
        vd = {0: [1, 2], 1: [0, 1, 2], 2: [0, 1]}[cd]
        vh = {0: [1, 2], 1: [0, 1, 2], 2: [0, 1]}[ch]
        vw = {0: [1, 2], 1: [0, 1, 2], 2: [0, 1]}[cw]
        return wdw[:, vd][:, :, vh][:, :, :, vw].sum(axis=(1, 2, 3))

    g = np.zeros((9, 64, 64), np.float32)
    g[0] = 1.0
    g[1, 0, :] = 1.0
    g[2, 63, :] = 1.0
    g[3, :, 0] = 1.0
    g[4, :, 63] = 1.0
    g[5, 0, 0] = 1.0
    g[6, 0, 63] = 1.0
    g[7, 63, 0] = 1.0
    g[8, 63, 63] = 1.0
    ind = g.reshape(9, HWC)

    def corr_for(dcase):
        c = np.zeros((9, 128), np.float32)
        base = Ssum(dcase, 1, 1)
        c[0] = base
        c[1] = Ssum(dcase, 0, 1) - base
        c[2] = Ssum(dcase, 2, 1) - base
        c[3] = Ssum(dcase, 1, 0) - base
        c[4] = Ssum(dcase, 1, 2) - base
        c[5] = Ssum(dcase, 0, 0) - Ssum(dcase, 0, 1) - Ssum(dcase, 1, 0) + base
        c[6] = Ssum(dcase, 0, 2) - Ssum(dcase, 0, 1) - Ssum(dcase, 1, 2) + base
        c[7] = Ssum(dcase, 2, 0) - Ssum(dcase, 2, 1) - Ssum(dcase, 1, 0) + base
        c[8] = Ssum(dcase, 2, 2) - Ssum(dcase, 2, 1) - Ssum(dcase, 1, 2) + base
        return c * b1e[None, :]

    corr_tab = {c: corr_for(c) for c in (0, 1, 2)}

    # conv fp8 scale: weights scaled so max ~ 120; inverse rides sd scale
    maxw = max(np.abs(wq).max(),
               max(np.abs(v).max() for v in corr_tab.values()), 1e-30)
    sc = int(np.floor(np.log2(120.0 / maxw)))
    wq8 = np.ascontiguousarray((wq * 2.0 ** sc).transpose(1, 0, 2, 3)).astype(f8)

    w4e = (w4 * ln2_w[None, :]).astype(np.float32)      # [FFN, C]
    b4e = b4 + w4 @ ln2_b
    w48 = np.zeros((65, 2, 128), np.float32)
    w48[0:64, 0, :] = w4e.T
    w48[64, 0, :] = -w4e.sum(axis=1)
    s4 = int(np.floor(np.log2(120.0 / max(np.abs(w48).max(), 1e-30))))
    w48_8 = (w48 * 2.0 ** s4).astype(f8)

    w5g = (w5 * gamma[:, None]).T                       # [FFN, C]
    s5 = int(np.floor(np.log2(120.0 / max(np.abs(w5g).max(), 1e-30))))
    s5 = min(s5, 24)
    w58 = np.zeros((128, 2, 64), np.float32)
    w58[:, 0, :] = w5g * 2.0 ** s5
    w58_8 = w58.astype(f8)

    ident2 = np.zeros((128, 64), np.float32)
    ident2[np.arange(64), np.arange(64)] = 2.0 ** s5
    ident2[64 + np.arange(64), np.arange(64)] = 2.0 ** s5
    bcl = np.where(np.abs(beta) < 1e-30, 1e-30, beta)
    identB = np.zeros((128, 64), np.float32)
    identB[np.arange(64), np.arange(64)] = 2.0 ** S3 / bcl
    identB[64 + np.arange(64), np.arange(64)] = 2.0 ** S3 / bcl

    b3b = b3 * beta
    common = dict(
        wq=wq8, ind=ind.astype(f8),
        modb=mod_b.reshape(128, 1).astype(np.float32),
        w3T=w3.T.astype(bf),
        bmap=np.broadcast_to(beta[None, :], (128, 64)).astype(bf).copy(),
        scawT=(sca_w.T / float(D * H * W)).astype(bf),
        scab=sca_b.reshape(128, 1).astype(np.float32),
        beta3=(beta * 2.0 ** -S3).reshape(64, 1).astype(np.float32),
        w48=w48_8, b4=b4e.reshape(128, 1).astype(np.float32),
        s4inv=np.full((128, 1), 2.0 ** -s4, np.float32),
        w58=w58_8, ident2=ident2.astype(bf),
        identB=identB.astype(bf),
        b5g=(b5 * gamma).reshape(64, 1).astype(np.float32),
        s5inv=np.full((64, 1), 2.0 ** -s5, np.float32),
        s5w3=np.full((128, 1), 2.0 ** s5, np.float32),
        i128=np.eye(128, dtype=np.float32).astype(bf),
        i64b=np.eye(64, dtype=np.float32).astype(bf),
    )

    in_maps = []
    for k in range(8):
        b, d0 = k // 4, (k % 4) * NPL
        ip = inp[b]                                     # [C, D, H, W]
        halo = np.zeros((NHALO, C, HWC), np.float32)
        lo, hi = max(d0 - 1, 0), min(d0 + NPL + 1, D)
        halo[lo - (d0 - 1):hi - (d0 - 1)] = (
            ip[:, lo:hi].transpose(1, 0, 2, 3).reshape(hi - lo, C, HWC))
        wcorr = np.zeros((9, 2, NPL, 128), np.float32)
        for i in range(NPL):
            dg = d0 + i
            dcase = 0 if dg == 0 else (2 if dg == D - 1 else 1)
            wcorr[:, 0, i, :] = corr_tab[dcase] * 2.0 ** sc
        m = dict(common)
        m["inp_t"] = halo.astype(bf)
        ipl = np.ascontiguousarray(
            ip[:, d0:d0 + NPL].transpose(1, 0, 2, 3).reshape(NPL, C, HWC))
        ipb = ipl + b3b[None, :, None]
        hi8 = ipb.astype(bf)
        lo8 = (ipb - hi8.astype(np.float32)).astype(bf)
        m["inp_hl"] = np.concatenate([hi8, lo8], axis=1)  # [NPL, 128, HWC]
        m["wc"] = wcorr.astype(f8)
        m["sd"] = (sdv[b] * 2.0 ** -sc).reshape(128, 1).astype(np.float32)
        in_maps.append(m)
    return in_maps


def kernel(**inputs):
    from concourse.bass_utils import run_bass_kernel_spmd
    if "nc" not in _CACHE:
        _CACHE["nc"] = _build()
    nc = _CACHE["nc"]
    in_maps = _host_prep(inputs)
    res = run_bass_kernel_spmd(nc, in_maps, list(range(8)))
    _CACHE["last_res"] = res
    out = np.empty((2, C, D, H, W), np.float32)
    for k in range(8):
        b, d0 = k // 4, (k % 4) * NPL
        o = res.results[k]["out"]                       # [NPL, C, HWC]
        out[b, :, d0:d0 + NPL] = o.reshape(NPL, C, H, W).transpose(1, 0, 2, 3)
    return out
